# revision 9
# baseline (speedup 1.0000x reference)
"""Multi-head attention (4x2048x1024, 16 heads) on 8 TRN2 NeuronCores.

Sharding: core c handles batch c//2, query seq-half c%2 (1024 queries).
Each core computes QKV projection for its own seq half plus K/V for the
peer half (redundant compute instead of a 2-rank collective), full
attention for all 16 heads over its 1024 queries x 2048 keys, and the
output projection. Outputs are disjoint -> no collectives; host concats.

Host passes transposed (d-major) shards with the core's own seq-half
first, so the SPMD graph is identical on every core.
"""

import numpy as np

import concourse.mybir as mybir
import concourse.tile as tile
from concourse import bacc
from concourse.bass_utils import run_bass_kernel_spmd

FP32 = mybir.dt.float32
BF16 = mybir.dt.bfloat16

DIM = 1024
HEADS = 16
HD = 64
AUG = HD + 1  # V columns per head + ones column for sum-exp
SCALE = DIM ** -0.5
SEQ = 2048
NI = 1024  # queries per core
NJ = 2048  # keys per core
B = 4
N_CORES = 8
P = 128

TRACE = False
LAST_RESULTS = None
_NC_CACHE = None


def _build():
    nc = bacc.Bacc(
        "TRN2",
        target_bir_lowering=False,
        debug=False,
        enable_asserts=False,
        num_devices=N_CORES,
    )
    xT = nc.dram_tensor("xT", [DIM, NJ], FP32, kind="ExternalInput")
    wqkvT = nc.dram_tensor("wqkvT", [DIM, 3 * DIM], FP32, kind="ExternalInput")
    woutT = nc.dram_tensor("woutT", [DIM, DIM], FP32, kind="ExternalInput")
    bout = nc.dram_tensor("bout", [1, DIM], FP32, kind="ExternalInput")
    out = nc.dram_tensor("out", [NI, DIM], FP32, kind="ExternalOutput")

    ND = DIM // P  # 8 contraction tiles

    with tile.TileContext(nc) as tc:
        with (
            tc.tile_pool(name="persist", bufs=1) as persist,
            tc.tile_pool(name="stage", bufs=3) as stage,
            tc.tile_pool(name="wpool", bufs=9) as wpool,
            tc.tile_pool(name="sb", bufs=3) as sb,
            tc.tile_pool(name="small", bufs=3) as small,
            tc.tile_pool(name="ps", bufs=4, space="PSUM") as psp,
        ):
            xpool_cm = tc.tile_pool(name="xpool", bufs=1)
            xpool = xpool_cm.__enter__()
            # ---- bias broadcast [1,1024] -> [128,1024]
            bias_sb = small.tile([1, DIM], FP32, tag="bias", name="bias", bufs=1)
            nc.sync.dma_start(out=bias_sb, in_=bout.ap())
            bias_bc = small.tile([P, DIM], FP32, tag="biasbc", name="biasbc", bufs=1)
            nc.gpsimd.partition_broadcast(bias_bc, bias_sb)

            # ---- load x^T and cast to bf16: 8 tiles [128 d, 2048 n]
            xbf = []
            for dt in range(ND):
                xb = xpool.tile([P, NJ], BF16, tag=f"xbf{dt}", name=f"xbf{dt}")
                for half in range(2):
                    xs = stage.tile([P, DIM], FP32, tag="stg", name="stg")
                    nc.sync.dma_start(
                        out=xs,
                        in_=xT.ap()[dt * P:(dt + 1) * P,
                                    half * DIM:(half + 1) * DIM],
                    )
                    dst = xb[:, half * DIM:(half + 1) * DIM]
                    if (dt + half) % 2 == 0:
                        nc.vector.tensor_copy(dst, xs)
                    else:
                        nc.scalar.copy(dst, xs)
                xbf.append(xb)

            def load_w_group(src_ap, ebase):
                """Load+cast 8 weight tiles [128 d, 1024 e] for one group."""
                grp = []
                for dt in range(ND):
                    ws = stage.tile([P, DIM], FP32, tag="stg", name="stg")
                    nc.sync.dma_start(
                        out=ws,
                        in_=src_ap[dt * P:(dt + 1) * P, ebase:ebase + DIM],
                    )
                    wb = wpool.tile([P, DIM], BF16, tag="wbf", name="wbf")
                    nc.vector.tensor_copy(wb, ws)
                    grp.append(wb)
                return grp

            # ---- Q/K projections: out e-major [e, n]
            qt = [persist.tile([P, NI], BF16, tag=f"qt{e}", name=f"qt{e}")
                  for e in range(8)]
            kt = [persist.tile([P, NJ], BF16, tag=f"kt{e}", name=f"kt{e}")
                  for e in range(8)]
            for g, (tiles, nchunks) in enumerate([(qt, NI // DIM), (kt, NJ // DIM)]):
                wg = load_w_group(wqkvT.ap(), g * DIM)
                for et in range(8):
                    for ch in range(nchunks):  # 1024-wide chunks
                        ps = psp.tile([P, DIM], FP32, tag="ps", name="ps")
                        for sc in range(2):  # 512-wide psum bank slices
                            nb = ch * DIM + sc * 512
                            for dt in range(ND):
                                nc.tensor.matmul(
                                    ps[:, sc * 512:(sc + 1) * 512],
                                    wg[dt][:, et * P:(et + 1) * P],
                                    xbf[dt][:, nb:nb + 512],
                                    start=(dt == 0),
                                    stop=(dt == ND - 1),
                                )
                        dst = tiles[et][:, ch * DIM:(ch + 1) * DIM]
                        if (et + ch) % 2 == 0:
                            nc.vector.tensor_copy(dst, ps)
                        else:
                            nc.scalar.copy(dst, ps)

            # ---- V projection: out n-major [n, e], scattered into 65-wide
            # per-head blocks with a ones column at offset 64 (sum-exp trick)
            vaug = [persist.tile([P, HEADS * AUG], BF16, tag=f"va{j}", name=f"va{j}")
                    for j in range(16)]
            for jt in range(16):
                v3 = vaug[jt].rearrange("p (h c) -> p h c", c=AUG)
                nc.vector.memset(v3[:, :, HD:AUG], 1.0)
            wg = load_w_group(wqkvT.ap(), 2 * DIM)
            for jt in range(16):
                ps = psp.tile([P, DIM], FP32, tag="ps", name="ps")
                for sc in range(2):  # e-chunks of 512 = 8 heads each
                    for dt in range(ND):
                        nc.tensor.matmul(
                            ps[:, sc * 512:(sc + 1) * 512],
                            xbf[dt][:, jt * P:(jt + 1) * P],
                            wg[dt][:, sc * 512:(sc + 1) * 512],
                            start=(dt == 0),
                            stop=(dt == ND - 1),
                        )
                src = ps.rearrange("p (h c) -> p h c", c=HD)
                dst = vaug[jt].rearrange("p (h c) -> p h c", c=AUG)[:, :, 0:HD]
                nc.scalar.copy(dst, src)

            xpool_cm.__exit__(None, None, None)
            norm_cm = tc.tile_pool(name="norm", bufs=3)
            norm = norm_cm.__enter__()

            # ---- attention, head pairs (2hp, 2hp+1) share e-tile hp
            aot = [persist.tile([P, NI], BF16, tag=f"ao{e}", name=f"ao{e}")
                   for e in range(8)]
            sume = small.tile([HEADS, NI], FP32, tag="sume", name="sume", bufs=1)
            for hp in range(8):
                avA = psp.tile([AUG, NI], FP32, tag="ps", name="av")
                avB = psp.tile([AUG, NI], FP32, tag="ps", name="av")
                for jt in range(16):
                    dA = psp.tile([P, NI], FP32, tag="ps", name="dots")
                    dB = psp.tile([P, NI], FP32, tag="ps", name="dots")
                    jsl = slice(jt * P, (jt + 1) * P)
                    for ic in range(2):
                        isl = slice(ic * 512, (ic + 1) * 512)
                        nc.tensor.matmul(
                            dA[:, isl], kt[hp][0:HD, jsl], qt[hp][0:HD, isl],
                            start=True, stop=True,
                        )
                    for ic in range(2):
                        isl = slice(ic * 512, (ic + 1) * 512)
                        nc.tensor.matmul(
                            dB[:, isl], kt[hp][HD:P, jsl], qt[hp][HD:P, isl],
                            start=True, stop=True,
                        )
                    eA = sb.tile([P, NI], BF16, tag="expT", name="expT", bufs=3)
                    nc.scalar.activation(eA, dA, mybir.ActivationFunctionType.Exp,
                                         scale=SCALE)
                    eB = sb.tile([P, NI], BF16, tag="expT", name="expT", bufs=3)
                    nc.scalar.activation(eB, dB, mybir.ActivationFunctionType.Exp,
                                         scale=SCALE)
                    first, last = jt == 0, jt == 15
                    for av, e_t, head in ((avA, eA, 2 * hp), (avB, eB, 2 * hp + 1)):
                        for ic in range(2):
                            isl = slice(ic * 512, (ic + 1) * 512)
                            nc.tensor.matmul(
                                av[:, isl],
                                vaug[jt][:, head * AUG:(head + 1) * AUG],
                                e_t[:, isl],
                                start=first, stop=last,
                            )
                # stash unnormalized out.T (bf16) + sum-exp rows; normalize
                # happens in one batched pass after all pairs. DVE writes must
                # start at 32-aligned partitions, so sum-exp rows go through a
                # base-0 temp and an SBUF->SBUF DMA into the gather tile.
                for av, head in ((avA, 2 * hp), (avB, 2 * hp + 1)):
                    row0 = (head % 2) * HD
                    nc.vector.tensor_copy(aot[hp][row0:row0 + HD, :], av[0:HD, :])
                    tse = norm.tile([1, NI], FP32, tag="tse", name="tse")
                    nc.vector.tensor_copy(tse, av[HD:AUG, :])
                    nc.sync.dma_start(out=sume[head:head + 1, :], in_=tse)

            # ---- batched softmax normalization: one exact reciprocal,
            # per-head partition-broadcast + in-place multiply
            recip = small.tile([HEADS, NI], FP32, tag="recip16", name="recip16",
                               bufs=1)
            nc.vector.reciprocal(recip, sume)
            for hp in range(8):
                for head in (2 * hp, 2 * hp + 1):
                    tr = norm.tile([1, NI], FP32, tag="tr", name="tr")
                    nc.sync.dma_start(out=tr, in_=recip[head:head + 1, :])
                    rb = norm.tile([P, NI], FP32, tag="rbc", name="rbc")
                    nc.gpsimd.partition_broadcast(rb, tr)
                    row0 = (head % 2) * HD
                    nc.vector.tensor_mul(
                        aot[hp][row0:row0 + HD, :],
                        aot[hp][row0:row0 + HD, :],
                        rb[row0:row0 + HD, :],
                    )

            norm_cm.__exit__(None, None, None)

            # ---- output projection + bias
            wo = load_w_group(woutT.ap(), 0)
            for it in range(8):
                ps = psp.tile([P, DIM], FP32, tag="ps", name="ps")
                for fc in range(2):
                    fsl = slice(fc * 512, (fc + 1) * 512)
                    for et in range(8):
                        nc.tensor.matmul(
                            ps[:, fsl],
                            aot[et][:, it * P:(it + 1) * P],
                            wo[et][:, fsl],
                            start=(et == 0),
                            stop=(et == 7),
                        )
                osb = sb.tile([P, DIM], FP32, tag="outsb", name="outsb", bufs=2)
                nc.vector.tensor_add(osb, ps, bias_bc)
                nc.sync.dma_start(out=out.ap()[it * P:(it + 1) * P, :], in_=osb)

    nc.compile()
    return nc


def _get_nc():
    global _NC_CACHE
    if _NC_CACHE is None:
        _NC_CACHE = _build()
    return _NC_CACHE


def kernel(x, w_qkv, w_out, b_out):
    global LAST_RESULTS
    x = np.asarray(x, dtype=np.float32)
    w_qkv = np.asarray(w_qkv, dtype=np.float32)
    w_out = np.asarray(w_out, dtype=np.float32)
    b_out = np.asarray(b_out, dtype=np.float32)

    nc = _get_nc()

    wqkvT = np.ascontiguousarray(w_qkv.T)
    woutT = np.ascontiguousarray(w_out.T)
    brow = np.ascontiguousarray(b_out.reshape(1, DIM))

    in_maps = []
    for c in range(N_CORES):
        b, h = divmod(c, 2)
        own = x[b, h * NI:(h + 1) * NI, :]
        peer = x[b, (1 - h) * NI:(2 - h) * NI, :]
        xTc = np.ascontiguousarray(np.concatenate([own, peer], axis=0).T)
        in_maps.append({
            "xT": xTc,
            "wqkvT": wqkvT,
            "woutT": woutT,
            "bout": brow,
        })

    res = run_bass_kernel_spmd(
        nc, in_maps, core_ids=list(range(N_CORES)), trace=TRACE
    )
    LAST_RESULTS = res

    out = np.empty((B, SEQ, DIM), dtype=np.float32)
    for c in range(N_CORES):
        b, h = divmod(c, 2)
        out[b, h * NI:(h + 1) * NI, :] = res.results[c]["out"]
    return out
